# revision 1
# baseline (speedup 1.0000x reference)
"""Trainium2 Bass kernel for CenterWoParamMultiCosineNearLoss.

loss = mean_b [ S_b - m_b + (2*m_b^2 - Q_b) / S_b ]   where, per sample b,
  d_k = 1 - <x_b, c_{label_b, k}>  (k = 0..15 sub-centers of own class)
  S = sum_k d_k, Q = sum_k d_k^2, m = min_k d_k
(algebraically identical to the reference's term1+term2; verified exactly).

Sharding: samples are sorted by label on the host (the loss is a mean over
samples, hence permutation-invariant) and split into 8 contiguous shards of
1024 — i.e. data-parallel with class-clustered assignment. Each core's shard
then spans only ~13 consecutive classes, so the device matmul per core is
x_shard^T [1024d x 1024b] against a small window of transposed centers
[1024d x 16*W cols] instead of all 90*16=1440 columns. Per-row selection of
the 16 own-class columns is done on device with an iota==label one-hot mask
and a reduction over the class axis (window columns are laid out k-major so
the reduce is contiguous). Row statistics (sum/sumsq/max over the 16 values)
are batched over all blocks at the end; each core emits its partial row-loss
sum as a [1,1] tensor and the host all-reduces the 8 scalars into the mean.
"""

import os
import sys

import numpy as np

for _p in ("/opt/trn_rl_repo", "/root/.axon_site/_ro/trn_rl_repo"):
    if os.path.isdir(_p) and _p not in sys.path:
        sys.path.append(_p)

import concourse.tile as tile  # noqa: E402
from concourse import bacc  # noqa: E402
from concourse import mybir  # noqa: E402
from concourse.bass_utils import run_bass_kernel_spmd  # noqa: E402

P = 128          # SBUF partitions
B = 8192         # batch
D = 1024         # feature dim
C = 90           # classes
K = 16           # sub-centers per class
NCORES = 8
SHARD = B // NCORES          # 1024 samples per core
NB = SHARD // P              # 8 row-blocks per core
KT = D // P                  # 8 contraction tiles

_F32 = mybir.dt.float32
_F16 = mybir.dt.float16

_ADD = mybir.AluOpType.add
_MULT = mybir.AluOpType.mult
_SUB = mybir.AluOpType.subtract
_MAX = mybir.AluOpType.max
_EQ = mybir.AluOpType.is_equal
_AX = mybir.AxisListType.X


def _build_program(wc: int):
    """One SPMD program for all 8 cores. wc = window width in columns (K*w)."""
    w = wc // K
    nc = bacc.Bacc(None, target_bir_lowering=False)
    xT = nc.declare_dram_parameter("xT", [NB // 2, P, KT, 2 * P], _F16, isOutput=False)
    cw = nc.declare_dram_parameter("cw", [P, KT, wc], _F16, isOutput=False)
    lab = nc.declare_dram_parameter("lab", [P, NB], _F32, isOutput=False)
    out = nc.declare_dram_parameter("out", [1, 1], _F32, isOutput=True)

    with tile.TileContext(nc) as tc:
        with (
            tc.tile_pool(name="const", bufs=1) as const,
            tc.tile_pool(name="cwp", bufs=1) as cwp,
            tc.tile_pool(name="xp", bufs=5) as xp,
            tc.tile_pool(name="maskp", bufs=NB) as maskp,
            tc.tile_pool(name="work", bufs=4) as work,
            tc.tile_pool(name="stats", bufs=1) as stats,
            tc.tile_pool(name="pp", bufs=6, space="PSUM") as pp,
            tc.tile_pool(name="ppf", bufs=1, space="PSUM") as ppf,
        ):
            # constants.  window columns are k-major: col j = k*w + c, so the
            # class id at column j is (j mod w)
            colc = const.tile([P, wc], _F32)
            nc.gpsimd.iota(
                colc[:, :], pattern=[[0, K], [1, w]], channel_multiplier=0,
                allow_small_or_imprecise_dtypes=True,
            )
            ones = const.tile([P, 1], _F32)
            nc.vector.memset(ones[:, :], 1.0)
            # small DMA via SWDGE (single queue/sem) — an HWDGE transfer fans
            # out across queues and overflows the consumer's wait slots
            labt = const.tile([P, NB], _F32)
            nc.gpsimd.dma_start(out=labt[:, :], in_=lab[:, :])

            # all 8 k-tiles of the centers window: [d_local, ktile, col]
            cwt = cwp.tile([P, KT, wc], _F16)
            nc.sync.dma_start(out=cwt[:, :, :], in_=cw[:, :, :])

            # one-hot masks for every block up-front (only depend on labt)
            masks = []
            for i in range(NB):
                mask = maskp.tile([P, wc], _F32)
                nc.vector.tensor_scalar(
                    out=mask[:, :], in0=colc[:, :], scalar1=labt[:, i : i + 1],
                    scalar2=None, op0=_EQ,
                )
                masks.append(mask)

            # selected cos values for every block: [p, block, k]
            dsel_all = stats.tile([P, NB, K], _F32)

            # packed two-block DMA chunks (4KB contiguous per partition); the
            # LAST pair is split into two k-tile halves so the final byte of
            # the stream only gates 8 matmuls instead of two whole blocks
            half = KT // 2
            xparts = []   # per pair: list of (tile, k_lo, k_hi)
            for j in range(NB // 2):
                if j < NB // 2 - 1:
                    xbt = xp.tile([P, KT, 2 * P], _F16, tag="xbt")
                    nc.sync.dma_start(out=xbt[:, :, :], in_=xT[j, :, :, :])
                    xparts.append([(xbt, 0, KT)])
                else:
                    parts = []
                    for klo in (0, half):
                        xh = xp.tile([P, half, 2 * P], _F16, tag="xbth")
                        nc.sync.dma_start(
                            out=xh[:, :, :], in_=xT[j, :, klo : klo + half, :]
                        )
                        parts.append((xh, klo, klo + half))
                    xparts.append(parts)
            for j in range(NB // 2):
                for h in range(2):
                    i = 2 * j + h
                    ps = pp.tile([P, wc], _F32)
                    for part, klo, khi in xparts[j]:
                        for k in range(klo, khi):
                            nc.tensor.matmul(
                                ps[:, :],
                                lhsT=part[:, k - klo, h * P : (h + 1) * P],
                                rhs=cwt[:, k, :],
                                start=(k == 0),
                                stop=(k == KT - 1),
                            )
                    sm = work.tile([P, wc], _F32)
                    nc.vector.tensor_tensor(
                        out=sm[:, :], in0=ps[:, :], in1=masks[i][:, :], op=_MULT
                    )
                    # collapse the class axis (contiguous, k-major layout);
                    # all other stats are batched over blocks at the end
                    nc.vector.tensor_reduce(
                        out=dsel_all[:, i, :],
                        in_=sm[:, :].rearrange("p (k c) -> p k c", c=w),
                        axis=_AX, op=_ADD,
                    )

            # batched per-row stats over all blocks: [P, NB] each
            ssum = stats.tile([P, NB], _F32)
            nc.vector.tensor_reduce(
                out=ssum[:, :], in_=dsel_all[:, :, :], axis=_AX, op=_ADD,
            )
            sqa = stats.tile([P, NB, K], _F32)
            nc.vector.tensor_tensor(
                out=sqa[:, :, :], in0=dsel_all[:, :, :], in1=dsel_all[:, :, :], op=_MULT
            )
            qsum = stats.tile([P, NB], _F32)
            nc.vector.tensor_reduce(
                out=qsum[:, :], in_=sqa[:, :, :], axis=_AX, op=_ADD,
            )
            mx = stats.tile([P, NB], _F32)
            nc.vector.tensor_reduce(
                out=mx[:, :], in_=dsel_all[:, :, :], axis=_AX, op=_MAX,
            )
            # epilogue on [P, NB]: d = 1 - s  =>
            #   S = K - ssum; Q = K - 2*ssum + qsum; m = 1 - mx
            #   rowloss = S - m + (2*m^2 - Q) / S
            # m-branch on the scalar engine, rest on vector (parallel)
            md = stats.tile([P, NB], _F32)
            nc.scalar.activation(
                out=md[:, :], in_=mx[:, :],
                func=mybir.ActivationFunctionType.Copy, bias=1.0, scale=-1.0,
            )
            num = stats.tile([P, NB], _F32)   # 2*m^2 = (sqrt(2)*m)^2
            nc.scalar.activation(
                out=num[:, :], in_=md[:, :],
                func=mybir.ActivationFunctionType.Square, scale=1.41421356237,
            )
            sd = stats.tile([P, NB], _F32)
            nc.vector.tensor_scalar(
                out=sd[:, :], in0=ssum[:, :], scalar1=-1.0, scalar2=float(K),
                op0=_MULT, op1=_ADD,
            )
            t = stats.tile([P, NB], _F32)
            nc.vector.tensor_scalar(
                out=t[:, :], in0=ssum[:, :], scalar1=-2.0, scalar2=float(K),
                op0=_MULT, op1=_ADD,
            )
            qd = stats.tile([P, NB], _F32)
            nc.vector.tensor_tensor(out=qd[:, :], in0=t[:, :], in1=qsum[:, :], op=_ADD)
            rs = stats.tile([P, NB], _F32)
            nc.vector.reciprocal(rs[:, :], sd[:, :])
            num2 = stats.tile([P, NB], _F32)
            nc.vector.tensor_tensor(out=num2[:, :], in0=num[:, :], in1=qd[:, :], op=_SUB)
            frac = stats.tile([P, NB], _F32)
            nc.vector.tensor_tensor(out=frac[:, :], in0=num2[:, :], in1=rs[:, :], op=_MULT)
            base = stats.tile([P, NB], _F32)
            nc.vector.tensor_tensor(out=base[:, :], in0=sd[:, :], in1=md[:, :], op=_SUB)
            rloss = stats.tile([P, NB], _F32)
            nc.vector.tensor_tensor(out=rloss[:, :], in0=base[:, :], in1=frac[:, :], op=_ADD)
            rowsum = stats.tile([P, 1], _F32)
            nc.vector.tensor_reduce(out=rowsum[:, :], in_=rloss[:, :], axis=_AX, op=_ADD)
            # cross-partition sum via ones-matmul -> single 4B output packet
            # (a [128,1] DMA shatters into 128 tiny packets and drains slowly)
            psc = ppf.tile([1, 1], _F32)
            nc.tensor.matmul(
                psc[:, :], lhsT=rowsum[:, :], rhs=ones[:, :], start=True, stop=True
            )
            outsb = stats.tile([1, 1], _F32)
            nc.vector.tensor_copy(out=outsb[:, :], in_=psc[:, :])
            nc.sync.dma_start(out=out[:, :], in_=outsb[:, :])

    nc.finalize()  # Bacc: runs wait-splitting + register allocation passes
    return nc


def _prep_inputs(x, labels, centers):
    """Host-side sharding/layout prep. Returns (in_maps, wc)."""
    labels = np.asarray(labels).astype(np.int64)
    x = np.ascontiguousarray(np.asarray(x, dtype=np.float32))
    centers = np.asarray(centers, dtype=np.float32)

    perm = np.argsort(labels, kind="stable")
    ls = labels[perm]

    # per-core class windows
    starts, spans = [], []
    for i in range(NCORES):
        seg = ls[i * SHARD : (i + 1) * SHARD]
        lo, hi = int(seg[0]), int(seg[-1])
        starts.append(lo)
        spans.append(hi - lo + 1)
    w = max(spans)
    assert w * K <= 512, f"class span {w} too large for single PSUM bank"
    wc = w * K
    starts = [min(s, C - w) for s in starts]

    centersT = np.ascontiguousarray(centers.reshape(C * K, D).T)  # [D, C*K]

    in_maps = []
    for i in range(NCORES):
        rows = perm[i * SHARD : (i + 1) * SHARD]
        # pack x into the exact SBUF tile layout so every DMA chunk is
        # contiguous per partition: [chunk, p, ktile, col]
        xsT = x[rows].T.astype(np.float16)                         # [D, SHARD]
        xdev = np.ascontiguousarray(
            xsT.reshape(KT, P, NB // 2, 2 * P).transpose(2, 1, 0, 3)
        )
        # window, k-major columns: col j = k*w + c  ->  centersT col 16*(start+c)+k
        win = centersT[:, K * starts[i] : K * (starts[i] + w)]     # [D, w*K] c-major
        cwin = win.reshape(D, w, K).transpose(0, 2, 1).reshape(D, wc)
        cwdev = np.ascontiguousarray(
            cwin.reshape(KT, P, wc).transpose(1, 0, 2).astype(np.float16)
        )
        lab_local = (ls[i * SHARD : (i + 1) * SHARD] - starts[i]).astype(np.float32)
        lab_dev = np.ascontiguousarray(lab_local.reshape(NB, P).T)  # [P, NB]
        in_maps.append({"xT": xdev, "cw": cwdev, "lab": lab_dev})
    return in_maps, wc


def kernel(x, labels, centers):
    in_maps, wc = _prep_inputs(x, labels, centers)
    nc = _build_program(wc)
    res = run_bass_kernel_spmd(nc, in_maps, core_ids=list(range(NCORES)))
    total = sum(float(r["out"].astype(np.float64).sum()) for r in res.results)
    return np.float32(total / B)



# revision 2
# speedup vs baseline: 1.1993x; 1.1993x over previous
"""Trainium2 Bass kernel for CenterWoParamMultiCosineNearLoss.

loss = mean_b [ 15 - s_b + x_b + N_b / D_b ]   where, per sample b,
  cos_k = <x_b, c_{label_b, k}>  (k = 0..15 sub-centers of own class)
  s = sum_k cos_k, q = sum_k cos_k^2, x = max_k cos_k
  D = 16 - s,  N = 2*(1-x)^2 - 16 + 2*s - q
(algebraically identical to the reference's term1+term2).

Sharding: samples are sorted by label on the host and split into 8
contiguous shards of 1024 (data-parallel, class-clustered).  All matmul
operands are fp8 e4m3 (validated: final-loss rel err ~1e-6) and use
DoubleRow perf mode (256-deep contraction per instruction).  Each
128-row block only multiplies against a G-class sub-window of its
core's center window; the per-block window offsets follow a fixed
schedule shared by all 8 cores (off_i computed from the data at build
time), so the single SPMD program stays valid for every core.  The
per-row one-hot selection masks are precomputed on the host and DMA'd
in, eliminating the label transfer / iota / compare chain.  Per-block
selection is one DVE multiply (psum x mask) written k-major, followed
by one batched fp16 reduce over the class axis.  Row statistics and the
loss epilogue run on DVE with the max-branch on the scalar engine; each
core emits one [1,1] partial that the host sums.
"""

import os
import sys

import numpy as np

for _p in ("/opt/trn_rl_repo", "/root/.axon_site/_ro/trn_rl_repo"):
    if os.path.isdir(_p) and _p not in sys.path:
        sys.path.append(_p)

import ml_dtypes  # noqa: E402

import concourse.tile as tile  # noqa: E402
from concourse import bacc  # noqa: E402
from concourse import mybir  # noqa: E402
from concourse.bass_utils import run_bass_kernel_spmd  # noqa: E402

P = 128          # SBUF partitions
B = 8192         # batch
D = 1024         # feature dim
C = 90           # classes
K = 16           # sub-centers per class
NCORES = 8
SHARD = B // NCORES          # 1024 samples per core
NB = SHARD // P              # 8 row-blocks per core
KT = D // P                  # 8 contraction tiles
KT2 = KT // 2                # 4 DoubleRow contraction pairs
NCH = NB // 2                # 4 x-chunks of 2 row-blocks

_F32 = mybir.dt.float32
_F16 = mybir.dt.float16
_F8 = mybir.dt.float8e4

_ADD = mybir.AluOpType.add
_MULT = mybir.AluOpType.mult
_SUB = mybir.AluOpType.subtract
_MAX = mybir.AluOpType.max
_AX = mybir.AxisListType.X
_DR = mybir.MatmulPerfMode.DoubleRow

F8NP = ml_dtypes.float8_e4m3


def _build_program(wc: int, gc: int, offs: list[int]):
    """One SPMD program for all 8 cores.

    wc   = core window width in columns (16 * classes)
    gc   = per-block window width in columns (16 * G)
    offs = per-block column offsets into the core window (shared schedule)
    """
    g = gc // K
    nc = bacc.Bacc(None, target_bir_lowering=False)
    xT = nc.declare_dram_parameter("xT", [NCH, P, KT, 2 * P], _F8, isOutput=False)
    cw = nc.declare_dram_parameter("cw", [P, KT, wc], _F8, isOutput=False)
    mk = nc.declare_dram_parameter("mk", [P, NB, gc], _F16, isOutput=False)
    out = nc.declare_dram_parameter("out", [1, 1], _F32, isOutput=True)

    with tile.TileContext(nc) as tc:
        with (
            tc.tile_pool(name="iop", bufs=1) as iop,
            tc.tile_pool(name="xp", bufs=NCH) as xp,
            tc.tile_pool(name="stats", bufs=1) as stats,
            tc.tile_pool(name="pp", bufs=6, space="PSUM") as pp,
            tc.tile_pool(name="ppf", bufs=1, space="PSUM") as ppf,
        ):
            ones = iop.tile([P, 1], _F32)
            nc.vector.memset(ones[:, :], 1.0)

            # --- DMA issues, spread over the two HWDGE engines -----------
            cwt = iop.tile([P, KT, wc], _F8)
            nc.sync.dma_start(out=cwt[:, :, :], in_=cw[:, :, :])
            xts = []
            for j in range(NCH):
                xbt = xp.tile([P, KT, 2 * P], _F8, tag="x")
                xts.append(xbt)
            nc.scalar.dma_start(out=xts[0][:, :, :], in_=xT[0, :, :, :])
            mkt = iop.tile([P, NB, gc], _F16)
            nc.scalar.dma_start(out=mkt[:, :, :], in_=mk[:, :, :])
            for j in range(1, NCH):
                nc.sync.dma_start(out=xts[j][:, :, :], in_=xT[j, :, :, :])

            # --- matmul + select per block -------------------------------
            # selected cos values for every block, k-major: [p, block, k, c]
            sel = stats.tile([P, NB, K, g], _F16)
            for j in range(NCH):
                for h in range(2):
                    i = 2 * j + h
                    o = offs[i] * K
                    ps = pp.tile([P, gc], _F32)
                    for t in range(KT2):
                        nc.tensor.matmul(
                            ps[:, :],
                            lhsT=xts[j][:, 2 * t : 2 * t + 2, h * P : (h + 1) * P],
                            rhs=cwt[:, 2 * t : 2 * t + 2, o : o + gc],
                            start=(t == 0),
                            stop=(t == KT2 - 1),
                            perf_mode=_DR,
                        )
                    # masked select; write transposed (c-major -> k-major)
                    nc.vector.tensor_tensor(
                        out=sel[:, i, :, :].rearrange("p k c -> p c k"),
                        in0=ps[:, :].rearrange("p (c k) -> p c k", k=K),
                        in1=mkt[:, i, :].rearrange("p (c k) -> p c k", k=K),
                        op=_MULT,
                    )

            # --- batched stats over all blocks ---------------------------
            dsel = stats.tile([P, NB, K], _F32)
            nc.vector.tensor_reduce(
                out=dsel[:, :, :], in_=sel[:, :, :, :], axis=_AX, op=_ADD
            )
            mx = stats.tile([P, NB], _F32)
            nc.vector.tensor_reduce(out=mx[:, :], in_=dsel[:, :, :], axis=_AX, op=_MAX)
            ssum = stats.tile([P, NB], _F32)
            nc.vector.tensor_reduce(out=ssum[:, :], in_=dsel[:, :, :], axis=_AX, op=_ADD)
            sq = stats.tile([P, NB, K], _F32)
            nc.vector.tensor_tensor(
                out=sq[:, :, :], in0=dsel[:, :, :], in1=dsel[:, :, :], op=_MULT
            )
            qsum = stats.tile([P, NB], _F32)
            nc.vector.tensor_reduce(out=qsum[:, :], in_=sq[:, :, :], axis=_AX, op=_ADD)

            # scalar-engine branch: md = 1 - x ; u2 = 2*md^2
            md = stats.tile([P, NB], _F32)
            nc.scalar.activation(
                out=md[:, :], in_=mx[:, :],
                func=mybir.ActivationFunctionType.Copy, bias=1.0, scale=-1.0,
            )
            u2 = stats.tile([P, NB], _F32)
            nc.scalar.activation(
                out=u2[:, :], in_=md[:, :],
                func=mybir.ActivationFunctionType.Square, scale=1.41421356237,
            )
            # DVE branch:
            #   Dd = 16 - s ; rs = 1/Dd ; e = 2s - 16 ; f = e - q
            #   N = u2 + f ; frac = N * rs ; h = (x - s) + frac
            sd = stats.tile([P, NB], _F32)
            nc.vector.tensor_scalar(
                out=sd[:, :], in0=ssum[:, :], scalar1=-1.0, scalar2=float(K),
                op0=_MULT, op1=_ADD,
            )
            rs = stats.tile([P, NB], _F32)
            nc.vector.reciprocal(rs[:, :], sd[:, :])
            e = stats.tile([P, NB], _F32)
            nc.vector.tensor_scalar(
                out=e[:, :], in0=ssum[:, :], scalar1=2.0, scalar2=-float(K),
                op0=_MULT, op1=_ADD,
            )
            f = stats.tile([P, NB], _F32)
            nc.vector.tensor_tensor(out=f[:, :], in0=e[:, :], in1=qsum[:, :], op=_SUB)
            nf = stats.tile([P, NB], _F32)
            nc.vector.tensor_tensor(out=nf[:, :], in0=u2[:, :], in1=f[:, :], op=_ADD)
            frac = stats.tile([P, NB], _F32)
            nc.vector.tensor_tensor(out=frac[:, :], in0=nf[:, :], in1=rs[:, :], op=_MULT)
            gg = stats.tile([P, NB], _F32)
            nc.vector.tensor_tensor(out=gg[:, :], in0=mx[:, :], in1=ssum[:, :], op=_SUB)
            hh = stats.tile([P, NB], _F32)
            nc.vector.tensor_tensor(out=hh[:, :], in0=gg[:, :], in1=frac[:, :], op=_ADD)
            rowsum = stats.tile([P, 1], _F32)
            nc.vector.tensor_reduce(out=rowsum[:, :], in_=hh[:, :], axis=_AX, op=_ADD)
            # cross-partition sum via ones-matmul -> single 4B output packet
            psc = ppf.tile([1, 1], _F32)
            nc.tensor.matmul(
                psc[:, :], lhsT=rowsum[:, :], rhs=ones[:, :], start=True, stop=True
            )
            outsb = stats.tile([1, 1], _F32)
            nc.vector.tensor_copy(out=outsb[:, :], in_=psc[:, :])
            nc.sync.dma_start(out=out[:, :], in_=outsb[:, :])

    nc.finalize()
    return nc


def _prep_inputs(x, labels, centers):
    """Host-side sharding/layout prep. Returns (in_maps, wc, gc, offs)."""
    labels = np.asarray(labels).astype(np.int64)
    x = np.ascontiguousarray(np.asarray(x, dtype=np.float32))
    centers = np.asarray(centers, dtype=np.float32)

    perm = np.argsort(labels, kind="stable")
    ls = labels[perm]

    # per-core window start = first class of the shard (no clamping; the
    # center matrix is zero-padded on the right so windows may run past C)
    starts = [int(ls[m * SHARD]) for m in range(NCORES)]
    # fixed per-block offset schedule shared by all cores
    offs, g = [], 0
    for i in range(NB):
        lo, hi = C, -1
        for m in range(NCORES):
            seg = ls[m * SHARD + i * P : m * SHARD + (i + 1) * P]
            lo = min(lo, int(seg[0]) - starts[m])
            hi = max(hi, int(seg[-1]) - starts[m])
        offs.append(lo)
        g = max(g, hi - lo + 1)
    w = max(o + g for o in offs)
    gc, wc = g * K, w * K
    assert gc <= 512, f"block class span {g} too large for a PSUM bank"

    centersT = centers.reshape(C * K, D).T          # [D, C*K] class-major
    pad = max(0, max(starts) + w - C)
    if pad:
        centersT = np.concatenate(
            [centersT, np.zeros((D, pad * K), np.float32)], axis=1
        )

    in_maps = []
    for m in range(NCORES):
        rows = perm[m * SHARD : (m + 1) * SHARD]
        xsT = x[rows].T.astype(F8NP)                               # [D, SHARD]
        xdev = np.ascontiguousarray(
            xsT.reshape(KT, P, NCH, 2 * P).transpose(2, 1, 0, 3)
        )
        win = centersT[:, K * starts[m] : K * (starts[m] + w)]     # [D, wc]
        cwdev = np.ascontiguousarray(
            win.reshape(KT, P, wc).transpose(1, 0, 2).astype(F8NP)
        )
        lab_local = (ls[m * SHARD : (m + 1) * SHARD]).astype(np.int64)
        lab_pb = lab_local.reshape(NB, P).T - starts[m]            # [P, NB]
        # mask[p, i, c*K + k] = 1 if offs[i] + c == local label of row
        cix = np.asarray(offs)[None, :, None] + np.arange(g)[None, None, :]
        mkdev = (cix == lab_pb[:, :, None]).astype(np.float16)     # [P, NB, g]
        mkdev = np.ascontiguousarray(
            np.repeat(mkdev[:, :, :, None], K, axis=3).reshape(P, NB, gc)
        )
        assert mkdev.reshape(P, NB, g, K)[:, :, :, 0].sum() == SHARD
        in_maps.append({"xT": xdev, "cw": cwdev, "mk": mkdev})
    return in_maps, wc, gc, offs


def kernel(x, labels, centers):
    in_maps, wc, gc, offs = _prep_inputs(x, labels, centers)
    nc = _build_program(wc, gc, offs)
    res = run_bass_kernel_spmd(nc, in_maps, core_ids=list(range(NCORES)))
    total = sum(float(r["out"].astype(np.float64).sum()) for r in res.results)
    return np.float32((total + 15.0 * B) / B)


# revision 4
# speedup vs baseline: 1.2057x; 1.0053x over previous
"""Trainium2 Bass kernel for CenterWoParamMultiCosineNearLoss.

loss = mean_b [ 15 - s_b + x_b + N_b / D_b ]   where, per sample b,
  cos_k = <x_b, c_{label_b, k}>  (k = 0..15 sub-centers of own class)
  s = sum_k cos_k, q = sum_k cos_k^2, x = max_k cos_k
  D = 16 - s,  N = 2*(1-x)^2 - 16 + 2*s - q
(algebraically identical to the reference's term1+term2).

Sharding: samples are sorted by label on the host and split into 8
contiguous shards of 1024 (data-parallel, class-clustered).  All matmul
operands are fp8 e4m3 (validated: final-loss rel err ~1e-6) and use
DoubleRow perf mode (256-deep contraction per instruction).  Each
128-row block multiplies against a G-class sub-window of its core's
center window; the per-block offsets follow a fixed schedule shared by
all 8 cores, computed from the data at build time.  One-hot selection
masks are precomputed on the host.  DMA is spread over both HWDGE rings
(sync: x blocks 0-3; scalar: centers, masks, x blocks 4-7) with no
scalar-engine activations so no act-table load competes with the scalar
ring.  Selection is one DVE multiply per block (psum x mask, written
k-major); class-collapse + row stats run in two halves so the first
half hides under the matmul phase.  The loss epilogue is split between
DVE and GpSimd; each core emits one [1,1] partial that the host sums.
"""

import os
import sys

import numpy as np

for _p in ("/opt/trn_rl_repo", "/root/.axon_site/_ro/trn_rl_repo"):
    if os.path.isdir(_p) and _p not in sys.path:
        sys.path.append(_p)

import ml_dtypes  # noqa: E402

import concourse.tile as tile  # noqa: E402
from concourse import bacc  # noqa: E402
from concourse import mybir  # noqa: E402
from concourse.bass_utils import run_bass_kernel_spmd  # noqa: E402

P = 128          # SBUF partitions
B = 8192         # batch
D = 1024         # feature dim
C = 90           # classes
K = 16           # sub-centers per class
NCORES = 8
SHARD = B // NCORES          # 1024 samples per core
NB = SHARD // P              # 8 row-blocks per core
KT = D // P                  # 8 contraction tiles
KT2 = KT // 2                # 4 DoubleRow contraction pairs
NH = NB // 2                 # blocks per stats half

_F32 = mybir.dt.float32
_F16 = mybir.dt.float16
_F8 = mybir.dt.float8e4

_ADD = mybir.AluOpType.add
_MULT = mybir.AluOpType.mult
_SUB = mybir.AluOpType.subtract
_MAX = mybir.AluOpType.max
_AX = mybir.AxisListType.X
_DR = mybir.MatmulPerfMode.DoubleRow

F8NP = ml_dtypes.float8_e4m3


def _build_program(wc: int, gc: int, offs: list[int]):
    """One SPMD program for all 8 cores.

    wc   = core window width in columns (16 * classes)
    gc   = per-block window width in columns (16 * G)
    offs = per-block column offsets into the core window (shared schedule)
    """
    g = gc // K
    nc = bacc.Bacc(None, target_bir_lowering=False)
    xa = nc.declare_dram_parameter("xa", [P, KT, 2 * P], _F8, isOutput=False)
    xb = nc.declare_dram_parameter("xb", [P, KT, 2 * P], _F8, isOutput=False)
    xcd = nc.declare_dram_parameter("xcd", [P, KT, 4 * P], _F8, isOutput=False)
    cw = nc.declare_dram_parameter("cw", [P, KT, wc], _F8, isOutput=False)
    mk = nc.declare_dram_parameter("mk", [P, NB, gc], _F16, isOutput=False)
    out = nc.declare_dram_parameter("out", [1, 1], _F32, isOutput=True)

    with tile.TileContext(nc) as tc:
        with (
            tc.tile_pool(name="iop", bufs=1) as iop,
            tc.tile_pool(name="stats", bufs=1) as stats,
            tc.tile_pool(name="pp", bufs=6, space="PSUM") as pp,
            tc.tile_pool(name="ppf", bufs=1, space="PSUM") as ppf,
        ):
            ones = iop.tile([P, 1], _F32)
            nc.vector.memset(ones[:, :], 1.0)

            # --- DMA issues: scalar ring carries cw+mk+xcd, sync ring xa+xb
            cwt = iop.tile([P, KT, wc], _F8)
            nc.scalar.dma_start(out=cwt[:, :, :], in_=cw[:, :, :])
            xat = iop.tile([P, KT, 2 * P], _F8)
            nc.sync.dma_start(out=xat[:, :, :], in_=xa[:, :, :])
            mkt = iop.tile([P, NB, gc], _F16)
            nc.scalar.dma_start(out=mkt[:, :, :], in_=mk[:, :, :])
            xbt = iop.tile([P, KT, 2 * P], _F8)
            nc.sync.dma_start(out=xbt[:, :, :], in_=xb[:, :, :])
            xcdt = iop.tile([P, KT, 4 * P], _F8)
            nc.scalar.dma_start(out=xcdt[:, :, :], in_=xcd[:, :, :])

            def xsrc(i):  # (tile, column offset) for block i's lhsT
                if i < 2:
                    return xat, i * P
                if i < 4:
                    return xbt, (i - 2) * P
                return xcdt, (i - 4) * P

            # selected cos values per block, k-major: [p, block, k, c]
            sel = stats.tile([P, NB, K, g], _F16)
            dsel = stats.tile([P, NB, K], _F16)
            mx = stats.tile([P, NB], _F16)
            ssum = stats.tile([P, NB], _F16)
            qsum = stats.tile([P, NB], _F16)

            def half_stats(ha):
                s0 = ha * NH
                sl = slice(s0, s0 + NH)
                # fp16 partials are safe here: |cos|<=1, 16-term sums, and the
                # final loss tolerance is 2e-2 (measured end-to-end ~1e-6)
                with nc.allow_low_precision(reason="fp16 stats, loose tolerance"):
                    nc.vector.tensor_reduce(
                        out=dsel[:, sl, :], in_=sel[:, sl, :, :], axis=_AX, op=_ADD
                    )
                    nc.vector.tensor_reduce(
                        out=mx[:, sl], in_=dsel[:, sl, :], axis=_AX, op=_MAX
                    )
                    nc.vector.tensor_reduce(
                        out=ssum[:, sl], in_=dsel[:, sl, :], axis=_AX, op=_ADD
                    )
                    sq = stats.tile([P, NH, K], _F16, tag=f"sq{ha}")
                    nc.vector.tensor_tensor(
                        out=sq[:, :, :], in0=dsel[:, sl, :], in1=dsel[:, sl, :],
                        op=_MULT,
                    )
                    nc.vector.tensor_reduce(
                        out=qsum[:, sl], in_=sq[:, :, :], axis=_AX, op=_ADD
                    )

            for i in range(NB):
                xt, xo = xsrc(i)
                o = offs[i] * K
                ps = pp.tile([P, gc], _F32)
                for t in range(KT2):
                    nc.tensor.matmul(
                        ps[:, :],
                        lhsT=xt[:, 2 * t : 2 * t + 2, xo : xo + P],
                        rhs=cwt[:, 2 * t : 2 * t + 2, o : o + gc],
                        start=(t == 0),
                        stop=(t == KT2 - 1),
                        perf_mode=_DR,
                    )
                # masked select; write transposed (c-major -> k-major)
                nc.vector.tensor_tensor(
                    out=sel[:, i, :, :].rearrange("p k c -> p c k"),
                    in0=ps[:, :].rearrange("p (c k) -> p c k", k=K),
                    in1=mkt[:, i, :].rearrange("p (c k) -> p c k", k=K),
                    op=_MULT,
                )
                if i == NH - 1:
                    half_stats(0)
            half_stats(1)

            # --- epilogue: DVE + GpSimd in parallel ----------------------
            #   gpsimd: md = 1-x ; u2 = md^2 ; n2 = 2*u2 ; e = 2s-16 ; gg = x-s
            #   DVE:    Dd = 16-s ; rs = 1/Dd ; f = e-q ; N = n2+f
            #           frac = N*rs ; h = gg+frac ; rowsum = sum(h)
            md = stats.tile([P, NB], _F32)
            nc.gpsimd.tensor_scalar(
                out=md[:, :], in0=mx[:, :], scalar1=-1.0, scalar2=1.0,
                op0=_MULT, op1=_ADD,
            )
            e = stats.tile([P, NB], _F32)
            nc.gpsimd.tensor_scalar(
                out=e[:, :], in0=ssum[:, :], scalar1=2.0, scalar2=-float(K),
                op0=_MULT, op1=_ADD,
            )
            gg = stats.tile([P, NB], _F32)
            nc.gpsimd.tensor_tensor(out=gg[:, :], in0=mx[:, :], in1=ssum[:, :], op=_SUB)
            u2 = stats.tile([P, NB], _F32)
            nc.gpsimd.tensor_tensor(out=u2[:, :], in0=md[:, :], in1=md[:, :], op=_MULT)
            n2 = stats.tile([P, NB], _F32)
            nc.gpsimd.tensor_scalar(
                out=n2[:, :], in0=u2[:, :], scalar1=2.0, scalar2=None, op0=_MULT,
            )
            sd = stats.tile([P, NB], _F32)
            nc.vector.tensor_scalar(
                out=sd[:, :], in0=ssum[:, :], scalar1=-1.0, scalar2=float(K),
                op0=_MULT, op1=_ADD,
            )
            rs = stats.tile([P, NB], _F32)
            nc.vector.reciprocal(rs[:, :], sd[:, :])
            f = stats.tile([P, NB], _F32)
            nc.vector.tensor_tensor(out=f[:, :], in0=e[:, :], in1=qsum[:, :], op=_SUB)
            nf = stats.tile([P, NB], _F32)
            nc.vector.tensor_tensor(out=nf[:, :], in0=n2[:, :], in1=f[:, :], op=_ADD)
            frac = stats.tile([P, NB], _F32)
            nc.vector.tensor_tensor(out=frac[:, :], in0=nf[:, :], in1=rs[:, :], op=_MULT)
            hh = stats.tile([P, NB], _F32)
            nc.vector.tensor_tensor(out=hh[:, :], in0=gg[:, :], in1=frac[:, :], op=_ADD)
            rowsum = stats.tile([P, 1], _F32)
            nc.vector.tensor_reduce(out=rowsum[:, :], in_=hh[:, :], axis=_AX, op=_ADD)
            # cross-partition sum via ones-matmul -> single 4B output packet
            psc = ppf.tile([1, 1], _F32)
            nc.tensor.matmul(
                psc[:, :], lhsT=rowsum[:, :], rhs=ones[:, :], start=True, stop=True
            )
            outsb = stats.tile([1, 1], _F32)
            nc.vector.tensor_copy(out=outsb[:, :], in_=psc[:, :])
            nc.sync.dma_start(out=out[:, :], in_=outsb[:, :])

    nc.finalize()
    return nc


def _prep_inputs(x, labels, centers):
    """Host-side sharding/layout prep. Returns (in_maps, wc, gc, offs)."""
    labels = np.asarray(labels).astype(np.int64)
    x = np.ascontiguousarray(np.asarray(x, dtype=np.float32))
    centers = np.asarray(centers, dtype=np.float32)

    perm = np.argsort(labels, kind="stable")
    ls = labels[perm]

    # per-core window start = first class of the shard (no clamping; the
    # center matrix is zero-padded on the right so windows may run past C)
    starts = [int(ls[m * SHARD]) for m in range(NCORES)]
    # fixed per-block offset schedule shared by all cores
    offs, g = [], 0
    for i in range(NB):
        lo, hi = C, -1
        for m in range(NCORES):
            seg = ls[m * SHARD + i * P : m * SHARD + (i + 1) * P]
            lo = min(lo, int(seg[0]) - starts[m])
            hi = max(hi, int(seg[-1]) - starts[m])
        offs.append(lo)
        g = max(g, hi - lo + 1)
    w = max(o + g for o in offs)
    gc, wc = g * K, w * K
    assert gc <= 512, f"block class span {g} too large for a PSUM bank"

    centersT = centers.reshape(C * K, D).T          # [D, C*K] class-major
    pad = max(0, max(starts) + w - C)
    if pad:
        centersT = np.concatenate(
            [centersT, np.zeros((D, pad * K), np.float32)], axis=1
        )

    in_maps = []
    for m in range(NCORES):
        rows = perm[m * SHARD : (m + 1) * SHARD]
        xsT = x[rows].T.astype(F8NP)                               # [D, SHARD]
        xfull = xsT.reshape(KT, P, SHARD).transpose(1, 0, 2)       # [P, KT, SHARD]
        xadev = np.ascontiguousarray(xfull[:, :, 0 : 2 * P])
        xbdev = np.ascontiguousarray(xfull[:, :, 2 * P : 4 * P])
        xcddev = np.ascontiguousarray(xfull[:, :, 4 * P : 8 * P])
        win = centersT[:, K * starts[m] : K * (starts[m] + w)]     # [D, wc]
        cwdev = np.ascontiguousarray(
            win.reshape(KT, P, wc).transpose(1, 0, 2).astype(F8NP)
        )
        lab_local = (ls[m * SHARD : (m + 1) * SHARD]).astype(np.int64)
        lab_pb = lab_local.reshape(NB, P).T - starts[m]            # [P, NB]
        # mask[p, i, c*K + k] = 1 if offs[i] + c == local label of row
        cix = np.asarray(offs)[None, :, None] + np.arange(g)[None, None, :]
        mkdev = (cix == lab_pb[:, :, None]).astype(np.float16)     # [P, NB, g]
        mkdev = np.ascontiguousarray(
            np.repeat(mkdev[:, :, :, None], K, axis=3).reshape(P, NB, gc)
        )
        assert mkdev.reshape(P, NB, g, K)[:, :, :, 0].sum() == SHARD
        in_maps.append(
            {"xa": xadev, "xb": xbdev, "xcd": xcddev, "cw": cwdev, "mk": mkdev}
        )
    return in_maps, wc, gc, offs


def kernel(x, labels, centers):
    in_maps, wc, gc, offs = _prep_inputs(x, labels, centers)
    nc = _build_program(wc, gc, offs)
    res = run_bass_kernel_spmd(nc, in_maps, core_ids=list(range(NCORES)))
    total = sum(float(r["out"].astype(np.float64).sum()) for r in res.results)
    return np.float32((total + 15.0 * B) / B)
